# revision 1
# baseline (speedup 1.0000x reference)
"""Trainium2 Bass kernel for BERT4ETH adjacency build:
    data = values * (features @ a0_weight[0])        # [E]
    out  = segment_sum(data, rows, num_segments=3M)  # [3M]

Distribution strategy (8 NeuronCores): shard by OUTPUT node range.
Each core owns a contiguous range of 376832 nodes (23 groups x 16384
nodes).  The host-side shard step routes each edge to the core/group
that owns its destination node (a coarse 184-bucket assignment); all
per-edge arithmetic -- the feature dot products, index decomposition,
one-hot construction, and the scatter-accumulate itself -- runs on
device.  Because output ranges are disjoint there is no all-reduce;
the host just concatenates the 8 per-core outputs.

Device algorithm, per core:
  for each of 23 groups (16384 nodes each, edges pre-bucketed):
    d = values * sum_f w_f * feat_f          (DVE, dense)
    p = m & 127 ; c = m >> 7                 (m = node idx within group)
    for each 128-edge tile:
      W[k,i] = (iota_i == p_k) * d_k         (one tensor_scalar, 4x mode, bf16)
      X[k,j] = (iota_j == c_k)               (one tensor_scalar, 4x mode, bf16)
      psum[c,p] += X^T @ W                   (TensorE, f32 accumulate)
    copy psum -> accum[:, g*128:(g+1)*128]
  one DMA: accum -> out  (out[g,c,p] = node g*16384 + c*128 + p)

Note: walrus embeds at most ONE sync-wait in a DVE/PE instruction, so the
structure below is arranged (primer ops + explicit order deps) so that every
compute instruction depends on at most one unobserved semaphore.
"""

import numpy as np
import ml_dtypes

import concourse.bass as bass
import concourse.mybir as mybir
from concourse.bass_utils import run_bass_kernel_spmd

F32 = mybir.dt.float32
BF16 = mybir.dt.bfloat16
I32 = mybir.dt.int32

N_CORES = 8
NUM_NODES = 3_000_000
GROUP_NODES = 16384          # nodes per group = 128*128 psum bins
N_GROUPS = 23                # groups per core
NODES_PER_CORE = N_GROUPS * GROUP_NODES   # 376832
TILES_PER_GROUP = 728        # 128-edge tiles per group (capacity 93184 edges)
EDGES_PER_GROUP = TILES_PER_GROUP * 128
N_FEAT = 5


def build_nc(n_groups=N_GROUPS, tiles_per_group=TILES_PER_GROUP, mask_ring=8,
             pe_check=4, repeat=1):
    """Build the per-core Bass program (same program on all 8 cores).

    Raw-bass (no Tile): this container's walrus only supports one embedded
    sync-wait per compute instruction, so all synchronization is standalone
    wait_ge instructions plus one then_inc per producing instruction.
    """
    ng, tg = n_groups, tiles_per_group
    cols = ng * tg  # free-dim length of the per-core edge arrays
    R = mask_ring

    nc = bass.Bass()

    feats = nc.dram_tensor("feats", [128, cols * N_FEAT], F32, kind="ExternalInput")
    vals = nc.dram_tensor("vals", [128, cols], F32, kind="ExternalInput")
    mloc = nc.dram_tensor("mloc", [128, cols], I32, kind="ExternalInput")
    wvec = nc.dram_tensor("wvec", [128, 8], F32, kind="ExternalInput")
    iota_in = nc.dram_tensor("iota", [128, 128], BF16, kind="ExternalInput")
    out = nc.dram_tensor("out", [ng, 128, 128], F32, kind="ExternalOutput")

    from contextlib import ExitStack
    ctx = ExitStack()
    with ctx:
        iota_sb = ctx.enter_context(nc.sbuf_tensor("iota_sb", [128, 128], BF16))
        w_sb = ctx.enter_context(nc.sbuf_tensor("w_sb", [128, 8], F32))
        c127 = ctx.enter_context(nc.sbuf_tensor("c127", [128, 1], I32))
        c7 = ctx.enter_context(nc.sbuf_tensor("c7", [128, 1], I32))
        accum = ctx.enter_context(nc.sbuf_tensor("accum", [128, ng * 128], F32))
        f_all = ctx.enter_context(nc.sbuf_tensor("f_sb", [128, 2 * tg * N_FEAT], F32))
        v_all = ctx.enter_context(nc.sbuf_tensor("v_sb", [128, 2 * tg], F32))
        m_all = ctx.enter_context(nc.sbuf_tensor("m_sb", [128, 2 * tg], I32))
        d_all = ctx.enter_context(nc.sbuf_tensor("d_sb", [128, 2 * tg], F32))
        pi_all = ctx.enter_context(nc.sbuf_tensor("pi_sb", [128, 2 * tg], I32))
        ci_all = ctx.enter_context(nc.sbuf_tensor("ci_sb", [128, 2 * tg], I32))
        pf_all = ctx.enter_context(nc.sbuf_tensor("pf_sb", [128, 2 * tg], F32))
        cf_all = ctx.enter_context(nc.sbuf_tensor("cf_sb", [128, 2 * tg], F32))
        wm_all = ctx.enter_context(nc.sbuf_tensor("wm_sb", [128, R * 128], BF16))
        xm_all = ctx.enter_context(nc.sbuf_tensor("xm_sb", [128, R * 128], BF16))
        f_sb = [f_all[:, i * tg * N_FEAT : (i + 1) * tg * N_FEAT] for i in range(2)]
        v_sb = [v_all[:, i * tg : (i + 1) * tg] for i in range(2)]
        m_sb = [m_all[:, i * tg : (i + 1) * tg] for i in range(2)]
        d_sb = [d_all[:, i * tg : (i + 1) * tg] for i in range(2)]
        pi_sb = [pi_all[:, i * tg : (i + 1) * tg] for i in range(2)]
        ci_sb = [ci_all[:, i * tg : (i + 1) * tg] for i in range(2)]
        pf_sb = [pf_all[:, i * tg : (i + 1) * tg] for i in range(2)]
        cf_sb = [cf_all[:, i * tg : (i + 1) * tg] for i in range(2)]
        wm_sb = [wm_all[:, i * 128 : (i + 1) * 128] for i in range(R)]
        xm_sb = [xm_all[:, i * 128 : (i + 1) * 128] for i in range(R)]
        psum0 = ctx.enter_context(nc.psum_tensor("psum0", [128, 128], F32))
        psum1 = ctx.enter_context(nc.psum_tensor("psum1", [128, 128], F32))
        s_din = ctx.enter_context(nc.semaphore("s_din"))
        s_prep = ctx.enter_context(nc.semaphore("s_prep"))
        s_mask = ctx.enter_context(nc.semaphore("s_mask"))
        s_pe = ctx.enter_context(nc.semaphore("s_pe"))
        s_evict = ctx.enter_context(nc.semaphore("s_evict"))
        s_dout = ctx.enter_context(nc.semaphore("s_dout"))
        block = ctx.enter_context(nc.Block())

        psums = [psum0, psum1]
        PREP_OPS = 11  # DVE prep ops per group (must match the vector block)

        def prep_end(g):
            return 2 + PREP_OPS * (g + 1)

        @block.sync
        def _(sync):
            sync.dma_start(out=iota_sb[:], in_=iota_in[:]).then_inc(s_din, 16)
            sync.dma_start(out=w_sb[:], in_=wvec[:]).then_inc(s_din, 16)
            for rep in range(repeat):
                for g in range(ng):
                    G = rep * ng + g
                    s = G % 2
                    if G >= 2:
                        # slot tenants from G-2 fully consumed after its prep
                        sync.wait_ge(s_prep, prep_end(G - 2))
                    sync.dma_start(
                        out=f_sb[s],
                        in_=feats[:, g * tg * N_FEAT : (g + 1) * tg * N_FEAT],
                    ).then_inc(s_din, 16)
                    sync.dma_start(
                        out=v_sb[s], in_=vals[:, g * tg : (g + 1) * tg]
                    ).then_inc(s_din, 16)
                    sync.dma_start(
                        out=m_sb[s], in_=mloc[:, g * tg : (g + 1) * tg]
                    ).then_inc(s_din, 16)
            sync.wait_ge(s_evict, ng * repeat)
            out_ap = bass.AP(out, 0, [[128, 128], [128 * 128, ng], [1, 128]])
            sync.dma_start(
                out=out_ap, in_=accum[:].rearrange("p (g q) -> p g q", g=ng)
            ).then_inc(s_dout, 16)
            sync.wait_ge(s_dout, 16)

        @block.vector
        def _(vector):
            # s_prep counts DVE prep-op completions (write-visibility guard:
            # a DVE op's writes are only guaranteed visible to a later DVE op
            # after a semaphore wait on the producer's completion).
            pcnt = 0

            def V(inst):
                nonlocal pcnt
                inst.then_inc(s_prep, 1)
                pcnt += 1

            def W():
                vector.wait_ge(s_prep, pcnt)

            V(nc.vector.memset(c127[:], 127))
            V(nc.vector.memset(c7[:], 7))
            vector.wait_ge(s_din, 32)  # iota + w
            for G in range(ng * repeat):
                g = G % ng
                s = G % 2
                vector.wait_ge(s_din, 32 + 48 * (G + 1))  # f,v,m of group g
                fg = f_sb[s]
                # d = values * sum_f w_f * feat_f
                V(nc.vector.tensor_copy(d_sb[s], fg[:, 0::N_FEAT]))
                W()
                V(nc.vector.tensor_tensor(
                    out=d_sb[s],
                    in0=d_sb[s],
                    in1=w_sb[:, 0:1].to_broadcast([128, tg]),
                    op=mybir.AluOpType.mult,
                ))
                for f in range(1, N_FEAT):
                    W()
                    V(nc.vector.scalar_tensor_tensor(
                        out=d_sb[s],
                        in0=fg[:, f::N_FEAT],
                        scalar=w_sb[:, f : f + 1],
                        in1=d_sb[s],
                        op0=mybir.AluOpType.mult,
                        op1=mybir.AluOpType.add,
                    ))
                W()
                V(nc.vector.tensor_tensor(
                    out=d_sb[s], in0=d_sb[s], in1=v_sb[s], op=mybir.AluOpType.mult
                ))
                # p = m & 127, c = m >> 7, as f32
                V(nc.vector.tensor_tensor(
                    out=pi_sb[s],
                    in0=m_sb[s],
                    in1=c127[:].to_broadcast([128, tg]),
                    op=mybir.AluOpType.bitwise_and,
                ))
                V(nc.vector.tensor_tensor(
                    out=ci_sb[s],
                    in0=m_sb[s],
                    in1=c7[:].to_broadcast([128, tg]),
                    op=mybir.AluOpType.logical_shift_right,
                ))
                W()
                V(nc.vector.tensor_copy(pf_sb[s], pi_sb[s]))
                V(nc.vector.tensor_copy(cf_sb[s], ci_sb[s]))
                assert pcnt == prep_end(G), (pcnt, G)
                W()  # all prep writes visible before the mask loop reads them
                for t in range(tg):
                    gt = G * tg + t  # global tile index
                    if gt >= R and t % pe_check == 0:
                        # mask ring slots for [gt, gt+pe_check) need matmuls
                        # up to gt - R + pe_check - 1 retired
                        vector.wait_ge(s_pe, gt - R + pe_check)
                    r = gt % R
                    nc.vector.tensor_scalar(
                        wm_sb[r],
                        iota_sb[:],
                        pf_sb[s][:, t : t + 1],
                        d_sb[s][:, t : t + 1],
                        mybir.AluOpType.is_equal,
                        mybir.AluOpType.mult,
                    )
                    nc.vector.tensor_scalar(
                        xm_sb[r],
                        iota_sb[:],
                        cf_sb[s][:, t : t + 1],
                        None,
                        mybir.AluOpType.is_equal,
                    ).then_inc(s_mask, 1)
                vector.wait_ge(s_pe, (G + 1) * tg)
                nc.vector.tensor_copy(
                    accum[:, g * 128 : (g + 1) * 128], psums[s][:]
                ).then_inc(s_evict, 1)

        @block.tensor
        def _(tensor):
            for G in range(ng * repeat):
                s = G % 2
                if G >= 2:
                    tensor.wait_ge(s_evict, G - 1)  # psum slot free
                for t in range(tg):
                    gt = G * tg + t
                    tensor.wait_ge(s_mask, gt + 1)
                    r = gt % R
                    nc.tensor.matmul(
                        out=psums[s][:],
                        lhsT=xm_sb[r],
                        rhs=wm_sb[r],
                        start=(t == 0),
                        stop=(t == tg - 1),
                    ).then_inc(s_pe, 1)

    return nc


# ---------------------------------------------------------------------------
# Host-side sharding / unsharding
# ---------------------------------------------------------------------------

def prepare_in_maps(features, values, rows, n_groups=N_GROUPS,
                    tiles_per_group=TILES_PER_GROUP):
    """Bucket edges by destination node into 8 cores x n_groups groups and
    lay each group out column-major in [128, tiles] tiles."""
    ng, tg = n_groups, tiles_per_group
    cols = ng * tg
    epg = tg * 128
    total_groups = N_CORES * ng

    features = np.asarray(features, dtype=np.float32)
    values = np.asarray(values, dtype=np.float32)
    rows = np.asarray(rows, dtype=np.int32)

    g_global = rows // GROUP_NODES  # [E] in [0, total_groups)
    order = np.argsort(g_global, kind="stable")
    g_sorted = g_global[order]
    counts = np.bincount(g_sorted, minlength=total_groups)
    if counts.max() > epg:
        raise RuntimeError(
            f"group overflow: max edges per group {counts.max()} > capacity {epg}"
        )
    starts = np.zeros(total_groups, dtype=np.int64)
    starts[1:] = np.cumsum(counts)[:-1]

    # destination flat position inside the owning core's [128, cols] array
    j_within = np.arange(len(rows), dtype=np.int64) - starts[g_sorted]
    g_local = (g_sorted % ng).astype(np.int64)
    pos = (j_within % 128) * cols + g_local * tg + (j_within // 128)
    core_of = (g_sorted // ng).astype(np.int64)
    gpos = core_of * (128 * cols) + pos  # position in a [8, 128, cols] array

    def scatter(src_sorted, fill=0.0, dtype=np.float32):
        dst = np.full(N_CORES * 128 * cols, fill, dtype=dtype)
        dst[gpos] = src_sorted
        return dst.reshape(N_CORES, 128, cols)

    vals_all = scatter(values[order])
    mloc_all = scatter((rows[order] - g_sorted * GROUP_NODES).astype(np.int32),
                       fill=0, dtype=np.int32)
    feats_flat = np.zeros((N_CORES * 128 * cols, N_FEAT), dtype=np.float32)
    feats_flat[gpos] = features[order]
    feats_all = feats_flat.reshape(N_CORES, 128, cols * N_FEAT)

    w8 = np.zeros(8, dtype=np.float32)
    return vals_all, mloc_all, feats_all, w8


def make_in_maps(features, values, a0_weight, rows,
                 n_groups=N_GROUPS, tiles_per_group=TILES_PER_GROUP):
    vals_all, mloc_all, feats_all, w8 = prepare_in_maps(
        features, values, rows, n_groups, tiles_per_group)
    w8[:N_FEAT] = np.asarray(a0_weight, dtype=np.float32).reshape(-1)[:N_FEAT]
    wvec = np.tile(w8[None, :], (128, 1)).astype(np.float32)
    iota = np.tile(np.arange(128, dtype=np.float32)[None, :], (128, 1)).astype(
        ml_dtypes.bfloat16
    )
    in_maps = []
    for c in range(N_CORES):
        in_maps.append({
            "feats": np.ascontiguousarray(feats_all[c]),
            "vals": np.ascontiguousarray(vals_all[c]),
            "mloc": np.ascontiguousarray(mloc_all[c]),
            "wvec": wvec,
            "iota": iota,
        })
    return in_maps


def timed_run(nc, in_maps, iters=5):
    """Run the kernel via PJRT with device-resident inputs and time executes.

    Returns (results_list, best_seconds). Wall-clock includes the axon RPC
    dispatch, so the min over iters is an upper bound on HW time.
    """
    import time
    import jax
    import concourse.mybir as _mybir
    from jax.sharding import Mesh, PartitionSpec, NamedSharding
    from jax.experimental.shard_map import shard_map
    from concourse import bass2jax as b2j

    b2j.install_neuronx_cc_hook()
    n_cores = len(in_maps)
    partition_name = nc.partition_id_tensor.name if nc.partition_id_tensor else None

    in_names, out_names, out_avals, zero_outs = [], [], [], []
    for alloc in nc.m.functions[0].allocations:
        if not isinstance(alloc, _mybir.MemoryLocationSet):
            continue
        name = alloc.memorylocations[0].name
        if alloc.kind == "ExternalInput":
            if name != partition_name:
                in_names.append(name)
        elif alloc.kind == "ExternalOutput":
            shape = tuple(alloc.tensor_shape)
            dtype = _mybir.dt.np(alloc.dtype)
            out_names.append(name)
            out_avals.append(jax.core.ShapedArray(shape, dtype))
            zero_outs.append(np.zeros(shape, dtype))
    n_params = len(in_names)
    all_in_names = list(in_names) + list(out_names)
    if partition_name is not None:
        all_in_names.append(partition_name)

    def _body(*args):
        operands = list(args)
        if partition_name is not None:
            operands.append(b2j.partition_id_tensor())
        outs = b2j._bass_exec_p.bind(
            *operands,
            out_avals=tuple(out_avals),
            in_names=tuple(all_in_names),
            out_names=tuple(out_names),
            lowering_input_output_aliases=(),
            sim_require_finite=True,
            sim_require_nnan=True,
            nc=nc,
        )
        return tuple(outs)

    devices = jax.devices()[:n_cores]
    mesh = Mesh(np.asarray(devices), ("core",))
    n_ops = n_params + len(out_names)
    fn = jax.jit(
        shard_map(
            _body,
            mesh=mesh,
            in_specs=(PartitionSpec("core"),) * n_ops,
            out_specs=(PartitionSpec("core"),) * len(out_names),
            check_rep=False,
        ),
        keep_unused=True,
    )
    concat_in = [
        np.concatenate([np.asarray(in_maps[c][nm]) for c in range(n_cores)], axis=0)
        for nm in in_names
    ]
    concat_zero = [
        np.zeros((n_cores * z.shape[0], *z.shape[1:]), z.dtype) for z in zero_outs
    ]
    sh = NamedSharding(mesh, PartitionSpec("core"))
    dev_args = [jax.device_put(x, sh) for x in concat_in + concat_zero]
    outs = fn(*dev_args)
    jax.block_until_ready(outs)
    best = float("inf")
    for _ in range(iters):
        t0 = time.perf_counter()
        outs = fn(*dev_args)
        jax.block_until_ready(outs)
        best = min(best, time.perf_counter() - t0)
    results = [
        {
            nm: np.asarray(outs[i]).reshape(n_cores, *out_avals[i].shape)[c]
            for i, nm in enumerate(out_names)
        }
        for c in range(n_cores)
    ]
    return results, best


_CACHE = {}


def kernel(features, values, a0_weight, rows, num_nodes):
    assert int(num_nodes) == NUM_NODES
    in_maps = make_in_maps(features, values, a0_weight, rows)
    if "nc" not in _CACHE:
        _CACHE["nc"] = build_nc()
    nc = _CACHE["nc"]
    res = run_bass_kernel_spmd(nc, in_maps, core_ids=list(range(N_CORES)))
    outs = [r["out"].reshape(-1) for r in res.results]
    full = np.concatenate(outs)[:NUM_NODES]
    return full.astype(np.float32)



# revision 2
# speedup vs baseline: 1.5815x; 1.5815x over previous
"""Trainium2 Bass kernel v3 for BERT4ETH adjacency build:
    data = values * (features @ a0_weight[0])        # [E]
    out  = segment_sum(data, rows, num_segments=3M)  # [3M]

FOLD design: no PE, no masks, no psum — DVE + DMA only, exact f32.

Host packing: nodes are classified by edge count c into depth classes
D = ceil(log2(max(c,1))) (width w = 2^D leaf slots, c <= 32).  Each
node owns w consecutive leaf columns in its class block; its edges
fill those slots (value 0 padding elsewhere).  The device computes
d = v * sum_f w_f f_f over all leaf slots, then reduces each node's
block with log2 in-place strided adds shared across the whole class:
    fold k:  X[:, off + 2^(k-1) :: 2^k]  +=  ...  (one tensor_tensor)
After D folds the node sums sit at stride-w positions; a strided copy
compacts them into the accumulator.  Output is a permutation of node
sums which the host inverts with one fancy-index gather.

Sharding: count-sorted nodes are dealt round-robin to 8 cores, then
round-robin to (chunk, partition) lanes inside each core, so class
populations are balanced everywhere; capacities are static with ~5%
margin and asserted at pack time.
"""

import numpy as np

import concourse.bass as bass
import concourse.mybir as mybir
from concourse.bass_utils import run_bass_kernel_spmd

F32 = mybir.dt.float32

N_CORES = 8
NUM_NODES = 3_000_000
N_FEAT = 5
NCHUNK = 12
MAXD = 6                      # depth classes 0..5 (c up to 32)
# per-(chunk, partition) node capacity of each class, tuned to the
# Poisson(5.59) count distribution of E=2^24 edges over 3M nodes
S_D = [7, 15, 66, 137, 29, 1]
W_D = [1 << d for d in range(MAXD)]
CLS_OFF = np.concatenate([[0], np.cumsum([w * s for w, s in zip(W_D, S_D)])])
K = int(CLS_OFF[-1])          # leaf columns per chunk
SUM_S = int(sum(S_D))         # accumulator columns per chunk
COLS = NCHUNK * K             # leaf columns per core
ACC = NCHUNK * SUM_S          # accumulator columns per core


def build_nc(nchunk=NCHUNK, repeat=1):
    nc = bass.Bass()
    feats = nc.dram_tensor("feats", [128, COLS * N_FEAT], F32, kind="ExternalInput")
    vals = nc.dram_tensor("vals", [128, COLS], F32, kind="ExternalInput")
    wvec = nc.dram_tensor("wvec", [128, 8], F32, kind="ExternalInput")
    out = nc.dram_tensor("out", [128, ACC], F32, kind="ExternalOutput")

    from contextlib import ExitStack
    ctx = ExitStack()
    with ctx:
        w_sb = ctx.enter_context(nc.sbuf_tensor("w_sb", [128, 8], F32))
        f_all = ctx.enter_context(nc.sbuf_tensor("f_sb", [128, 2 * K * N_FEAT], F32))
        v_all = ctx.enter_context(nc.sbuf_tensor("v_sb", [128, 2 * K], F32))
        d_all = ctx.enter_context(nc.sbuf_tensor("d_sb", [128, 2 * K], F32))
        accum = ctx.enter_context(nc.sbuf_tensor("accum", [128, ACC], F32))
        f_sb = [f_all[:, i * K * N_FEAT : (i + 1) * K * N_FEAT] for i in range(2)]
        v_sb = [v_all[:, i * K : (i + 1) * K] for i in range(2)]
        d_sb = [d_all[:, i * K : (i + 1) * K] for i in range(2)]
        s_const = ctx.enter_context(nc.semaphore("s_const"))
        s_din = ctx.enter_context(nc.semaphore("s_din"))
        s_prep = ctx.enter_context(nc.semaphore("s_prep"))
        s_done = ctx.enter_context(nc.semaphore("s_done"))
        s_dout = ctx.enter_context(nc.semaphore("s_dout"))
        block = ctx.enter_context(nc.Block())

        @block.sync
        def _(sync):
            sync.dma_start(out=w_sb[:], in_=wvec[:]).then_inc(s_const, 16)
            for cc in range(nchunk * repeat):
                c = cc % nchunk
                s = cc % 2
                if cc >= 2:
                    sync.wait_ge(s_done, cc - 1)
                sync.dma_start(
                    out=f_sb[s],
                    in_=feats[:, c * K * N_FEAT : (c + 1) * K * N_FEAT],
                ).then_inc(s_din, 16)
                sync.dma_start(
                    out=v_sb[s], in_=vals[:, c * K : (c + 1) * K]
                ).then_inc(s_din, 16)
            sync.wait_ge(s_done, nchunk * repeat)
            sync.dma_start(out=out[:], in_=accum[:]).then_inc(s_dout, 16)
            sync.wait_ge(s_dout, 16)

        @block.vector
        def _(vector):
            pcnt = 0

            def V(inst):
                nonlocal pcnt
                inst.then_inc(s_prep, 1)
                pcnt += 1

            def Wt():
                vector.wait_ge(s_prep, pcnt)

            vector.wait_ge(s_const, 16)
            for cc in range(nchunk * repeat):
                c = cc % nchunk
                s = cc % 2
                vector.wait_ge(s_din, 32 * (cc + 1))
                fg, X = f_sb[s], d_sb[s]
                # d = v * sum_f w_f * f_f
                V(nc.vector.tensor_scalar(
                    X, fg[:, 0::N_FEAT], w_sb[:, 0:1], None,
                    mybir.AluOpType.mult,
                ))
                for f in range(1, N_FEAT):
                    Wt()
                    V(nc.vector.scalar_tensor_tensor(
                        out=X, in0=fg[:, f::N_FEAT],
                        scalar=w_sb[:, f : f + 1], in1=X,
                        op0=mybir.AluOpType.mult, op1=mybir.AluOpType.add,
                    ))
                Wt()
                V(nc.vector.tensor_tensor(
                    out=X, in0=X, in1=v_sb[s], op=mybir.AluOpType.mult
                ))
                # folds, level k = 1..D for every class at once
                for k in range(1, MAXD):
                    Wt()
                    step = 1 << k
                    half = 1 << (k - 1)
                    for D in range(k, MAXD):
                        if S_D[D] == 0:
                            continue
                        off = int(CLS_OFF[D])
                        end = int(CLS_OFF[D + 1])
                        V(nc.vector.tensor_tensor(
                            out=X[:, off : end : step],
                            in0=X[:, off : end : step],
                            in1=X[:, off + half : end : step],
                            op=mybir.AluOpType.add,
                        ))
                # compact node sums into accum; last copy signals chunk done
                Wt()
                aoff = c * SUM_S
                acc_base = np.cumsum([0] + S_D)
                last_D = max(D for D in range(MAXD) if S_D[D] > 0)
                for D in range(MAXD):
                    if S_D[D] == 0:
                        continue
                    off = int(CLS_OFF[D])
                    end = int(CLS_OFF[D + 1])
                    so = aoff + int(acc_base[D])
                    inst = nc.vector.tensor_copy(
                        accum[:, so : so + S_D[D]], X[:, off : end : W_D[D]]
                    )
                    if D == last_D:
                        inst.then_inc(s_done, 1)
                    else:
                        inst.then_inc(s_prep, 1)
                        pcnt += 1

    return nc


# ---------------------------------------------------------------------------
# Host-side packing / unpacking
# ---------------------------------------------------------------------------

def _depth(c):
    d = np.zeros_like(c)
    for D in range(1, MAXD):
        d[c > (1 << (D - 1))] = D
    return d


def pack(rows):
    rows = np.asarray(rows, dtype=np.int64)
    counts = np.bincount(rows, minlength=NUM_NODES).astype(np.int64)
    if counts.max() > (1 << (MAXD - 1)):
        raise RuntimeError(f"node count {counts.max()} exceeds max class width")

    order = np.argsort(-counts, kind="stable")  # count-sorted node ranks
    node_core = np.empty(NUM_NODES, dtype=np.int64)
    node_core[order] = np.arange(NUM_NODES) % N_CORES

    depth = _depth(counts)

    # per (core, class): deal nodes round-robin over (chunk, partition) lanes
    node_chunk = np.empty(NUM_NODES, dtype=np.int64)
    node_part = np.empty(NUM_NODES, dtype=np.int64)
    node_idx = np.empty(NUM_NODES, dtype=np.int64)   # slot index within lane
    acc_base = np.cumsum([0] + S_D)
    for core in range(N_CORES):
        sel_core = node_core == core
        for D in range(MAXD):
            nodes = order[sel_core[order] & (depth[order] == D)]
            n = len(nodes)
            if n > S_D[D] * 128 * NCHUNK:
                raise RuntimeError(
                    f"core {core} class {D}: {n} nodes > capacity "
                    f"{S_D[D] * 128 * NCHUNK}"
                )
            r = np.arange(n)
            node_part[nodes] = r % 128
            node_chunk[nodes] = (r // 128) % NCHUNK
            node_idx[nodes] = r // (128 * NCHUNK)

    # leaf column of each node: chunk*K + CLS_OFF[D] + idx*w
    node_leafcol = (
        node_chunk * K + CLS_OFF[depth] + node_idx * np.array(W_D)[depth]
    )
    # accumulator column: chunk*SUM_S + acc_base[D] + idx
    node_acccol = node_chunk * SUM_S + np.array(acc_base)[depth] + node_idx

    # edge placement: j-th edge of node n -> leaf column node_leafcol[n]+j
    eorder = np.argsort(rows, kind="stable")
    rs = rows[eorder]
    starts = np.zeros(NUM_NODES, dtype=np.int64)
    starts[1:] = np.cumsum(counts)[:-1]
    j = np.arange(len(rows), dtype=np.int64) - starts[rs]
    ecol = node_leafcol[rs] + j
    epart = node_part[rs]
    gpos = node_core[rs] * (128 * COLS) + epart * COLS + ecol

    label = node_core * (128 * ACC) + node_part * ACC + node_acccol
    return eorder, gpos, label


def make_in_maps(features, values, a0_weight, rows):
    features = np.asarray(features, dtype=np.float32)
    values = np.asarray(values, dtype=np.float32)
    eorder, gpos, label = pack(rows)

    vals_all = np.zeros(N_CORES * 128 * COLS, dtype=np.float32)
    vals_all[gpos] = values[eorder]
    vals_all = vals_all.reshape(N_CORES, 128, COLS)

    feats_flat = np.zeros((N_CORES * 128 * COLS, N_FEAT), dtype=np.float32)
    feats_flat[gpos] = features[eorder]
    feats_all = feats_flat.reshape(N_CORES, 128, COLS * N_FEAT)

    w8 = np.zeros(8, dtype=np.float32)
    w8[:N_FEAT] = np.asarray(a0_weight, dtype=np.float32).reshape(-1)[:N_FEAT]
    wvec = np.tile(w8[None, :], (128, 1)).astype(np.float32)

    in_maps = []
    for c in range(N_CORES):
        in_maps.append({
            "feats": np.ascontiguousarray(feats_all[c]),
            "vals": np.ascontiguousarray(vals_all[c]),
            "wvec": wvec,
        })
    return in_maps, label


def unshard(results, label):
    outs = [np.asarray(r["out"]).reshape(-1) for r in results]
    full = np.concatenate(outs)
    return full[label].astype(np.float32)


_CACHE = {}


def kernel(features, values, a0_weight, rows, num_nodes):
    assert int(num_nodes) == NUM_NODES
    in_maps, label = make_in_maps(features, values, a0_weight, rows)
    if "nc" not in _CACHE:
        _CACHE["nc"] = build_nc()
    nc = _CACHE["nc"]
    res = run_bass_kernel_spmd(nc, in_maps, core_ids=list(range(N_CORES)))
    return unshard(res.results, label)
